# revision 5
# baseline (speedup 1.0000x reference)
"""Trainium2 Bass kernel for nn_Attention_27797028340174.

Multi-head attention, B=4, S=2048, H=16 heads, D=64 (HID=1024):
    x = query.reshape(B*S, HID)                     (the `key` input is
    q,k,v = x@Wq+bq, x@Wk+bk, x@Wv+bv                ignored: source bug
    per (b,h): softmax(q k^T / 8) @ v                makes k,v from query)

Sharding: tensor-parallel over the 16 heads -> 2 heads per NeuronCore,
zero collectives. Each core receives the full transposed activations
xT = x.T (bf16) plus its 128-column slice of Wq/Wk/Wv, and produces its
[8192, 128] slice of the output; the host concatenates slices.

Device algorithm per core (all matmuls bf16, fp32 PSUM):
  qT,kT = W.T @ xT        [64, 4096] per-head column blocks
  v     = xT.T @ Wv       [seq, 128] natural layout (+ ones column)
  per (b,h), per j-tile:  scoresT[j,i] = kT_tile.T @ qT  (K=64)
      expT = exp(scoresT/8)  (ScalarE, bf16 out)
  per i-tile: ctx[i, 0:64], Z[i] = expT_tiles.T @ [v | 1] (K=128 chain)
      out = ctx * reciprocal(Z)    (VectorE)

Assumptions hard-verified on host: attention_mask all ones (mask term
== 0), zero biases. These hold for the problem's setup_inputs().
"""

from contextlib import ExitStack

import numpy as np
import ml_dtypes

import concourse.bass as bass
import concourse.tile as tile
from concourse import bacc, mybir
from concourse.bass_utils import run_bass_kernel_spmd

BF16 = mybir.dt.bfloat16
F32 = mybir.dt.float32

B = 4  # batches
S = 2048  # seq per batch
HID = 1024
NCORES = 8
NH = 2  # heads per core
D = 64
KT = 8  # hid tiles of 128
JT = 16  # key tiles of 128 per batch
IT = 16  # query tiles of 128 per batch
CH = 4  # seq chunks of 512 per batch
CW = 512  # chunk width

EXP_BUFS = 28

_CACHE = {}


def _build():
    nc = bacc.Bacc(
        "TRN2", target_bir_lowering=False, debug=False, num_devices=NCORES
    )
    xt = nc.dram_tensor("xt", [HID, B * S], BF16, kind="ExternalInput")
    wq = nc.dram_tensor("wq", [HID, 128], BF16, kind="ExternalInput")
    wk = nc.dram_tensor("wk", [HID, 128], BF16, kind="ExternalInput")
    wv = nc.dram_tensor("wv", [HID, 128], BF16, kind="ExternalInput")
    out = nc.dram_tensor("out", [B * S, 128], F32, kind="ExternalOutput")

    xt_v = xt.ap().rearrange("(kt p) n -> p kt n", p=128)  # [128, 8, 8192]
    # out viewed [128p, b, it, c]
    out_v = out.ap().rearrange("(b it p) c -> p b it c", it=IT, p=128)

    with tile.TileContext(nc) as tc, ExitStack() as ctx:
        wp = ctx.enter_context(tc.tile_pool(name="w", bufs=1))
        xp = ctx.enter_context(tc.tile_pool(name="x", bufs=2))
        qkp = ctx.enter_context(tc.tile_pool(name="qk", bufs=2))
        ep = ctx.enter_context(tc.tile_pool(name="e", bufs=EXP_BUFS))
        op = ctx.enter_context(tc.tile_pool(name="o", bufs=2))
        zp = ctx.enter_context(tc.tile_pool(name="z", bufs=4))
        psq = ctx.enter_context(tc.tile_pool(name="psq", bufs=2, space="PSUM"))
        pss = ctx.enter_context(tc.tile_pool(name="pss", bufs=2, space="PSUM"))
        psc = ctx.enter_context(tc.tile_pool(name="psc", bufs=2, space="PSUM"))

        wq_sb = wp.tile([128, KT, 128], BF16)
        nc.sync.dma_start(wq_sb[:], wq.ap().rearrange("(kt p) m -> p kt m", p=128))
        wk_sb = wp.tile([128, KT, 128], BF16)
        nc.sync.dma_start(wk_sb[:], wk.ap().rearrange("(kt p) m -> p kt m", p=128))
        wv_sb = wp.tile([128, KT, 128], BF16)
        nc.sync.dma_start(wv_sb[:], wv.ap().rearrange("(kt p) m -> p kt m", p=128))

        # per-batch state, double buffered across batches
        state = {}

        def load_xt(b, ch):
            gc = b * CH + ch
            xt_t = xp.tile([128, KT, CW], BF16, tag="xt", name="xt_t")
            nc.sync.dma_start(xt_t[:], xt_v[:, :, gc * CW : (gc + 1) * CW])
            return xt_t

        def emit_qk_chunk(b, ch, which):
            """q or k projection for seq chunk ch (512 wide) of batch b."""
            st = state[b]
            xt_t = load_xt(b, ch)
            w_sb = wq_sb if which == "q" else wk_sb
            dst = st["qT"] if which == "q" else st["kT"]
            ps = psq.tile([128, CW], F32, tag="qkv", name="ps_qk")
            for kt in range(KT):
                nc.tensor.matmul(
                    ps[:],
                    lhsT=w_sb[:, kt],
                    rhs=xt_t[:, kt],
                    start=(kt == 0),
                    stop=(kt == KT - 1),
                )
            # head A (psum rows 0:64) straight to column block 0
            nc.vector.tensor_copy(
                out=dst[:, ch * CW : (ch + 1) * CW], in_=ps[0:64, :]
            )
            # head B (rows 64:128) -> staging, then partition-shift DMA
            stg = xp.tile([128, CW], BF16, tag="stg", name="stg")
            nc.vector.tensor_copy(out=stg[64:128, :], in_=ps[64:128, :])
            nc.sync.dma_start(
                dst[:, S + ch * CW : S + (ch + 1) * CW], stg[64:128, :]
            )

        def emit_v_chunk(b, ch):
            st = state[b]
            xt_t = load_xt(b, ch)
            for sub in range(4):
                jt = ch * 4 + sub
                ps = psq.tile([128, 128], F32, tag="qkv", name="ps_v")
                for kt in range(KT):
                    nc.tensor.matmul(
                        ps[:],
                        lhsT=xt_t[:, kt, sub * 128 : (sub + 1) * 128],
                        rhs=wv_sb[:, kt],
                        start=(kt == 0),
                        stop=(kt == KT - 1),
                    )
                nc.vector.tensor_copy(
                    out=st["v"][:, jt, :, 0:D],
                    in_=ps[:].rearrange("p (h d) -> p h d", h=NH),
                )

        def alloc_batch(b):
            st = {}
            st["qT"] = qkp.tile([64, NH * S], BF16, tag="qT", name="qT")
            st["kT"] = qkp.tile([64, NH * S], BF16, tag="kT", name="kT")
            st["v"] = qkp.tile([128, JT, NH, D + 1], BF16, tag="v", name="v")
            nc.vector.memset(st["v"][:, :, :, D], 1.0)
            state[b] = st

        def emit_scores(p, jt):
            """Scores + exp for pair p=(b,h), key tile jt. Returns expT."""
            b, h = divmod(p, NH)
            st = state[b]
            e = ep.tile([128, S], BF16, tag="e")
            for ihalf in range(2):
                ps = pss.tile([128, 1024], F32, tag="s")
                for ic in range(2):
                    i0 = ihalf * 1024 + ic * CW
                    nc.tensor.matmul(
                        ps[:, ic * CW : (ic + 1) * CW],
                        lhsT=st["kT"][:, h * S + jt * 128 : h * S + (jt + 1) * 128],
                        rhs=st["qT"][:, h * S + i0 : h * S + i0 + CW],
                        start=True,
                        stop=True,
                    )
                nc.scalar.activation(
                    e[:, ihalf * 1024 : (ihalf + 1) * 1024],
                    ps[:],
                    mybir.ActivationFunctionType.Exp,
                    scale=0.125,
                )
            return e

        def emit_ctx(p, it, exps, o_sb):
            b, h = divmod(p, NH)
            st = state[b]
            ps = psc.tile([128, D + 1], F32, tag="c")
            for jt in range(JT):
                nc.tensor.matmul(
                    ps[:],
                    lhsT=exps[jt][:, it * 128 : (it + 1) * 128],
                    rhs=st["v"][:, jt, h],
                    start=(jt == 0),
                    stop=(jt == JT - 1),
                )
            rz = zp.tile([128, 1], F32, tag="rz")
            nc.vector.reciprocal(rz[:], ps[:, D : D + 1])
            nc.vector.tensor_scalar_mul(o_sb[:, it], ps[:, 0:D], rz[:])

        def emit_out_dma(p, o_sb):
            b, h = divmod(p, NH)
            nc.sync.dma_start(out_v[:, b, :, h * D : (h + 1) * D], o_sb[:])

        # ---- prologue: ACT table warmup + first projections ----
        warm_src = wp.tile([128, 1], F32, name="warm_src")
        warm_dst = wp.tile([128, 1], F32, name="warm_dst")
        nc.vector.memset(warm_src[:], 0.0)
        nc.scalar.activation(
            warm_dst[:], warm_src[:], mybir.ActivationFunctionType.Exp
        )
        alloc_batch(0)
        for ch in range(CH):
            emit_qk_chunk(0, ch, "q")
        emit_qk_chunk(0, 0, "k")

        # ---- pipelined pairs ----
        NP = B * NH
        prev = None  # (pair index, exps list)
        for p in range(NP):
            b, h = divmod(p, NH)
            exps = []
            o_prev = (
                op.tile([128, IT, D], F32, tag="o", name="o_sb")
                if prev is not None
                else None
            )
            for jt in range(JT):
                exps.append(emit_scores(p, jt))
                if prev is not None:
                    # two ctx i-tiles per early slot -> expT released by jt=8
                    if jt < 8:
                        emit_ctx(prev[0], 2 * jt, prev[1], o_prev)
                        emit_ctx(prev[0], 2 * jt + 1, prev[1], o_prev)
                    if jt == 8:
                        emit_out_dma(prev[0], o_prev)
                # interleaved projection work, spread to keep ScalarE fed
                if p == 0:
                    if jt in (0, 2, 4):
                        emit_qk_chunk(0, 1 + jt // 2, "k")
                    elif jt in (9, 11, 13, 15):
                        emit_v_chunk(0, (jt - 9) // 2)
                elif h == 1 and b + 1 < B:
                    if jt == 1:
                        alloc_batch(b + 1)
                    if jt in (1, 3, 5, 7):
                        emit_qk_chunk(b + 1, (jt - 1) // 2, "q")
                    elif jt in (9, 11, 13, 15):
                        emit_qk_chunk(b + 1, (jt - 9) // 2, "k")
                elif h == 0 and b >= 1:
                    if jt in (2, 6, 10, 14):
                        emit_v_chunk(b, (jt - 2) // 4)
            prev = (p, exps)

        # ---- epilogue: ctx for the last pair ----
        o_last = op.tile([128, IT, D], F32, tag="o")
        for it in range(IT):
            emit_ctx(prev[0], it, prev[1], o_last)
        emit_out_dma(prev[0], o_last)

    nc.compile()
    return nc


def _get_nc():
    if "nc" not in _CACHE:
        _CACHE["nc"] = _build()
    return _CACHE["nc"]


def kernel(
    query,
    key=None,
    attention_mask=None,
    Wq=None,
    bq=None,
    Wk=None,
    bk=None,
    Wv=None,
    bv=None,
    seq_length=2048,
    **_unused,
):
    query = np.asarray(query)
    Wq = np.asarray(Wq)
    Wk = np.asarray(Wk)
    Wv = np.asarray(Wv)
    if attention_mask is not None and not np.all(np.asarray(attention_mask) == 1):
        raise NotImplementedError("kernel assumes an all-ones attention mask")
    for bias in (bq, bk, bv):
        if bias is not None and np.any(np.asarray(bias)):
            raise NotImplementedError("kernel assumes zero biases")

    x = query.reshape(-1, HID)  # [8192, 1024]
    xt = np.ascontiguousarray(x.T).astype(ml_dtypes.bfloat16)  # [1024, 8192]

    in_maps = []
    for c in range(NCORES):
        cols = slice(c * 128, (c + 1) * 128)
        in_maps.append(
            {
                "xt": xt,
                "wq": np.ascontiguousarray(Wq[:, cols]).astype(ml_dtypes.bfloat16),
                "wk": np.ascontiguousarray(Wk[:, cols]).astype(ml_dtypes.bfloat16),
                "wv": np.ascontiguousarray(Wv[:, cols]).astype(ml_dtypes.bfloat16),
            }
        )

    nc = _get_nc()
    res = run_bass_kernel_spmd(
        nc,
        in_maps,
        core_ids=list(range(NCORES)),
        trace=bool(_CACHE.get("trace", False)),
    )
    _CACHE["last_result"] = res
    out = np.concatenate(
        [res.results[c]["out"] for c in range(NCORES)], axis=1
    ).astype(np.float32)
    return out


# revision 10
# speedup vs baseline: 1.0763x; 1.0763x over previous
"""Trainium2 Bass kernel for nn_Attention_27797028340174.

Multi-head attention, B=4, S=2048, H=16 heads, D=64 (HID=1024):
    x = query.reshape(B*S, HID)                     (the `key` input is
    q,k,v = x@Wq+bq, x@Wk+bk, x@Wv+bv                ignored: source bug
    per (b,h): softmax(q k^T / 8) @ v                makes k,v from query)

Sharding: tensor-parallel over the 16 heads -> 2 heads per NeuronCore,
zero collectives. Each core receives the full transposed activations
xT = x.T (bf16) plus its 128-column slice of Wq/Wk/Wv, and produces its
[8192, 128] slice of the output; the host concatenates slices.

Device algorithm per core (all matmuls bf16, fp32 PSUM):
  qT,kT = W.T @ xT        [64, 4096] per-head column blocks
  v     = xT.T @ Wv       [seq, 128] natural layout (+ ones column)
  per (b,h), per j-tile:  scoresT[j,i] = kT_tile.T @ qT  (K=64)
      expT = exp(scoresT/8)  (ScalarE, bf16 out)
  per i-tile: ctx[i, 0:64], Z[i] = expT_tiles.T @ [v | 1] (K=128 chain)
      out = ctx * reciprocal(Z)    (VectorE)

Assumptions hard-verified on host: attention_mask all ones (mask term
== 0), zero biases. These hold for the problem's setup_inputs().
"""

from contextlib import ExitStack

import numpy as np
import ml_dtypes

import concourse.bass as bass
import concourse.tile as tile
from concourse import bacc, mybir
from concourse.bass_utils import run_bass_kernel_spmd

BF16 = mybir.dt.bfloat16
F32 = mybir.dt.float32

B = 4  # batches
S = 2048  # seq per batch
HID = 1024
NCORES = 8
NH = 2  # heads per core
D = 64
KT = 8  # hid tiles of 128
JT = 16  # key tiles of 128 per batch
IT = 16  # query tiles of 128 per batch
CH = 4  # seq chunks of 512 per batch
CW = 512  # chunk width

EXP_BUFS = 27
XT_BUFS = 3

_CACHE = {}


def _build():
    nc = bacc.Bacc(
        "TRN2", target_bir_lowering=False, debug=False, num_devices=NCORES
    )
    xt = nc.dram_tensor("xt", [HID, B * S], BF16, kind="ExternalInput")
    wq = nc.dram_tensor("wq", [HID, 128], BF16, kind="ExternalInput")
    wk = nc.dram_tensor("wk", [HID, 128], BF16, kind="ExternalInput")
    wv = nc.dram_tensor("wv", [HID, 128], BF16, kind="ExternalInput")
    out = nc.dram_tensor("out", [B * S, 128], F32, kind="ExternalOutput")

    xt_v = xt.ap().rearrange("(kt p) n -> p kt n", p=128)  # [128, 8, 8192]
    # out viewed [128p, b, it, c]
    out_v = out.ap().rearrange("(b it p) c -> p b it c", it=IT, p=128)

    with tile.TileContext(nc) as tc, ExitStack() as ctx:
        wp = ctx.enter_context(tc.tile_pool(name="w", bufs=1))
        xp = ctx.enter_context(tc.tile_pool(name="x", bufs=XT_BUFS))
        qkp = ctx.enter_context(tc.tile_pool(name="qk", bufs=2))
        ep = ctx.enter_context(tc.tile_pool(name="e", bufs=EXP_BUFS))
        op = ctx.enter_context(tc.tile_pool(name="o", bufs=2))
        zp = ctx.enter_context(tc.tile_pool(name="z", bufs=4))
        psq = ctx.enter_context(tc.tile_pool(name="psq", bufs=2, space="PSUM"))
        pss = ctx.enter_context(tc.tile_pool(name="pss", bufs=2, space="PSUM"))
        psc = ctx.enter_context(tc.tile_pool(name="psc", bufs=2, space="PSUM"))

        wq_sb = wp.tile([128, KT, 128], BF16)
        nc.sync.dma_start(wq_sb[:], wq.ap().rearrange("(kt p) m -> p kt m", p=128))
        wk_sb = wp.tile([128, KT, 128], BF16)
        nc.sync.dma_start(wk_sb[:], wk.ap().rearrange("(kt p) m -> p kt m", p=128))
        wv_sb = wp.tile([128, KT, 128], BF16)
        nc.sync.dma_start(wv_sb[:], wv.ap().rearrange("(kt p) m -> p kt m", p=128))

        # per-batch state, double buffered across batches
        state = {}

        def emit_qkv_chunk(b, ch):
            """Projections for seq chunk ch (512 wide) of batch b."""
            st = state[b]
            gc = b * CH + ch
            xt_t = xp.tile([128, KT, CW], BF16, tag="xt", name="xt_t")
            nc.sync.dma_start(xt_t[:], xt_v[:, :, gc * CW : (gc + 1) * CW])
            for w_sb, dst in ((wq_sb, st["qT"]), (wk_sb, st["kT"])):
                ps = psq.tile([128, CW], F32, tag="qkv", name="ps_qk")
                for kt in range(KT):
                    nc.tensor.matmul(
                        ps[:],
                        lhsT=w_sb[:, kt],
                        rhs=xt_t[:, kt],
                        start=(kt == 0),
                        stop=(kt == KT - 1),
                    )
                # head A (psum rows 0:64) straight to column block 0
                nc.vector.tensor_copy(
                    out=dst[:, ch * CW : (ch + 1) * CW], in_=ps[0:64, :]
                )
                # head B (rows 64:128) -> staging, then partition-shift DMA
                stg = xp.tile([128, CW], BF16, tag="stg", name="stg")
                nc.vector.tensor_copy(out=stg[64:128, :], in_=ps[64:128, :])
                nc.sync.dma_start(
                    dst[:, S + ch * CW : S + (ch + 1) * CW], stg[64:128, :]
                )
            for sub in range(4):
                jt = ch * 4 + sub
                ps = psq.tile([128, 128], F32, tag="qkv", name="ps_v")
                for kt in range(KT):
                    nc.tensor.matmul(
                        ps[:],
                        lhsT=xt_t[:, kt, sub * 128 : (sub + 1) * 128],
                        rhs=wv_sb[:, kt],
                        start=(kt == 0),
                        stop=(kt == KT - 1),
                    )
                nc.vector.tensor_copy(
                    out=st["v"][:, jt, :, 0:D],
                    in_=ps[:].rearrange("p (h d) -> p h d", h=NH),
                )

        def alloc_batch(b):
            st = {}
            st["qT"] = qkp.tile([64, NH * S], BF16, tag="qT", name="qT")
            st["kT"] = qkp.tile([64, NH * S], BF16, tag="kT", name="kT")
            st["v"] = qkp.tile([128, JT, NH, D + 1], BF16, tag="v", name="v")
            nc.vector.memset(st["v"][:, :, :, D], 1.0)
            state[b] = st

        def emit_scores(p, jt, halves=(0, 1), e=None):
            """Scores + exp for pair p=(b,h), key tile jt. Returns expT."""
            b, h = divmod(p, NH)
            st = state[b]
            if e is None:
                e = ep.tile([128, S], BF16, tag="e", name="e")
            for ihalf in halves:
                ps = pss.tile([128, 1024], F32, tag="s")
                for ic in range(2):
                    i0 = ihalf * 1024 + ic * CW
                    nc.tensor.matmul(
                        ps[:, ic * CW : (ic + 1) * CW],
                        lhsT=st["kT"][:, h * S + jt * 128 : h * S + (jt + 1) * 128],
                        rhs=st["qT"][:, h * S + i0 : h * S + i0 + CW],
                        start=True,
                        stop=True,
                    )
                nc.scalar.activation(
                    e[:, ihalf * 1024 : (ihalf + 1) * 1024],
                    ps[:],
                    mybir.ActivationFunctionType.Exp,
                    scale=0.125,
                )
            return e

        def emit_ctx(p, it, exps, o_sb):
            b, h = divmod(p, NH)
            st = state[b]
            ps = psc.tile([128, D + 1], F32, tag="c")
            for jt in range(JT):
                nc.tensor.matmul(
                    ps[:],
                    lhsT=exps[jt][:, it * 128 : (it + 1) * 128],
                    rhs=st["v"][:, jt, h],
                    start=(jt == 0),
                    stop=(jt == JT - 1),
                )
            rz = zp.tile([128, 1], F32, tag="rz")
            nc.vector.reciprocal(rz[:], ps[:, D : D + 1])
            nc.vector.tensor_scalar_mul(o_sb[:, it], ps[:, 0:D], rz[:])

        def emit_out_dma(p, o_sb):
            b, h = divmod(p, NH)
            nc.sync.dma_start(out_v[:, b, :, h * D : (h + 1) * D], o_sb[:])

        # ---- prologue: ACT table warmup + batch-0 projections with early
        # partial scores (i-half 0 of jt 0-3 only needs q chunks 0-1 + k ch0)
        warm_src = wp.tile([128, 1], F32, name="warm_src")
        warm_dst = wp.tile([128, 1], F32, name="warm_dst")
        nc.vector.memset(warm_src[:], 0.0)
        nc.scalar.activation(
            warm_dst[:], warm_src[:], mybir.ActivationFunctionType.Exp
        )
        alloc_batch(0)
        emit_qkv_chunk(0, 0)
        emit_qkv_chunk(0, 1)
        exps0 = [emit_scores(0, jt, halves=(0,)) for jt in range(4)]
        emit_qkv_chunk(0, 2)
        emit_qkv_chunk(0, 3)
        for jt in range(4):
            emit_scores(0, jt, halves=(1,), e=exps0[jt])
        for jt in range(4, JT):
            exps0.append(emit_scores(0, jt))
        prev = (0, exps0)

        # ---- pipelined pairs ----
        NP = B * NH
        for p in range(1, NP):
            b, h = divmod(p, NH)
            exps = []
            o_prev = op.tile([128, IT, D], F32, tag="o", name="o_sb")
            for jt in range(JT):
                exps.append(emit_scores(p, jt))
                # two ctx i-tiles per early slot -> expT released by jt=8
                if jt < 8:
                    emit_ctx(prev[0], 2 * jt, prev[1], o_prev)
                    emit_ctx(prev[0], 2 * jt + 1, prev[1], o_prev)
                if jt == 8:
                    emit_out_dma(prev[0], o_prev)
                # interleave next batch's projections into the h=1 pair
                if h == 1 and b + 1 < B and jt in (2, 6, 10, 14):
                    if jt == 2:
                        alloc_batch(b + 1)
                    emit_qkv_chunk(b + 1, jt // 4)
            prev = (p, exps)

        # ---- epilogue: ctx for the last pair ----
        o_last = op.tile([128, IT, D], F32, tag="o")
        for it in range(IT):
            emit_ctx(prev[0], it, prev[1], o_last)
        emit_out_dma(prev[0], o_last)

    nc.compile()
    return nc


def _get_nc():
    if "nc" not in _CACHE:
        _CACHE["nc"] = _build()
    return _CACHE["nc"]


def kernel(
    query,
    key=None,
    attention_mask=None,
    Wq=None,
    bq=None,
    Wk=None,
    bk=None,
    Wv=None,
    bv=None,
    seq_length=2048,
    **_unused,
):
    query = np.asarray(query)
    Wq = np.asarray(Wq)
    Wk = np.asarray(Wk)
    Wv = np.asarray(Wv)
    if attention_mask is not None and not np.all(np.asarray(attention_mask) == 1):
        raise NotImplementedError("kernel assumes an all-ones attention mask")
    for bias in (bq, bk, bv):
        if bias is not None and np.any(np.asarray(bias)):
            raise NotImplementedError("kernel assumes zero biases")

    x = query.reshape(-1, HID)  # [8192, 1024]
    xt = np.ascontiguousarray(x.T).astype(ml_dtypes.bfloat16)  # [1024, 8192]

    in_maps = []
    for c in range(NCORES):
        cols = slice(c * 128, (c + 1) * 128)
        in_maps.append(
            {
                "xt": xt,
                "wq": np.ascontiguousarray(Wq[:, cols]).astype(ml_dtypes.bfloat16),
                "wk": np.ascontiguousarray(Wk[:, cols]).astype(ml_dtypes.bfloat16),
                "wv": np.ascontiguousarray(Wv[:, cols]).astype(ml_dtypes.bfloat16),
            }
        )

    nc = _get_nc()
    res = run_bass_kernel_spmd(
        nc,
        in_maps,
        core_ids=list(range(NCORES)),
        trace=bool(_CACHE.get("trace", False)),
    )
    _CACHE["last_result"] = res
    out = np.concatenate(
        [res.results[c]["out"] for c in range(NCORES)], axis=1
    ).astype(np.float32)
    return out


# revision 17
# speedup vs baseline: 1.1426x; 1.0616x over previous
"""Trainium2 Bass kernel for nn_Attention_27797028340174.

Multi-head attention, B=4, S=2048, H=16 heads, D=64 (HID=1024):
    x = query.reshape(B*S, HID)                     (the `key` input is
    q,k,v = x@Wq+bq, x@Wk+bk, x@Wv+bv                ignored: source bug
    per (b,h): softmax(q k^T / 8) @ v                makes k,v from query)

Sharding: tensor-parallel over the 16 heads -> 2 heads per NeuronCore,
zero collectives. Each core receives the full transposed activations
xT = x.T (bf16) plus its 128-column slice of Wq/Wk/Wv, and produces its
[8192, 128] slice of the output; the host concatenates slices.

Device algorithm per core (all matmuls bf16, fp32 PSUM):
  qT,kT = W.T @ xT        [64, 4096] per-head column blocks
  v     = xT.T @ Wv       [seq, 128] natural layout (+ ones column)
  per (b,h), per j-tile:  scoresT[j,i] = kT_tile.T @ qT  (K=64)
      expT = exp(scoresT/8)  (ScalarE, bf16 out)
  per i-tile: ctx[i, 0:64], Z[i] = expT_tiles.T @ [v | 1] (K=128 chain)
      out = ctx * reciprocal(Z)    (VectorE)

Assumptions hard-verified on host: attention_mask all ones (mask term
== 0), zero biases. These hold for the problem's setup_inputs().
"""

from contextlib import ExitStack

import numpy as np
import ml_dtypes

import concourse.bass as bass
import concourse.tile as tile
from concourse import bacc, mybir
from concourse.bass_utils import run_bass_kernel_spmd

BF16 = mybir.dt.bfloat16
F32 = mybir.dt.float32

B = 4  # batches
S = 2048  # seq per batch
HID = 1024
NCORES = 8
NH = 2  # heads per core
D = 64
KT = 8  # hid tiles of 128
JT = 16  # key tiles of 128 per batch
IT = 16  # query tiles of 128 per batch
CH = 4  # seq chunks of 512 per batch
CW = 512  # chunk width

EXP_BUFS = 27
XT_BUFS = 3

_CACHE = {}


def _build():
    nc = bacc.Bacc(
        "TRN2", target_bir_lowering=False, debug=False, num_devices=NCORES
    )
    xt = nc.dram_tensor("xt", [HID, B * S], BF16, kind="ExternalInput")
    wq = nc.dram_tensor("wq", [HID, 128], BF16, kind="ExternalInput")
    wk = nc.dram_tensor("wk", [HID, 128], BF16, kind="ExternalInput")
    wv = nc.dram_tensor("wv", [HID, 128], BF16, kind="ExternalInput")
    out = nc.dram_tensor("out", [B * S, 128], F32, kind="ExternalOutput")

    xt_v = xt.ap().rearrange("(kt p) n -> p kt n", p=128)  # [128, 8, 8192]
    # out viewed [128p, b, it, c]
    out_v = out.ap().rearrange("(b it p) c -> p b it c", it=IT, p=128)

    with tile.TileContext(nc) as tc, ExitStack() as ctx:
        wp = ctx.enter_context(tc.tile_pool(name="w", bufs=1))
        xp = ctx.enter_context(tc.tile_pool(name="x", bufs=XT_BUFS))
        qkp = ctx.enter_context(tc.tile_pool(name="qk", bufs=2))
        ep = ctx.enter_context(tc.tile_pool(name="e", bufs=EXP_BUFS))
        op = ctx.enter_context(tc.tile_pool(name="o", bufs=2))
        zp = ctx.enter_context(tc.tile_pool(name="z", bufs=4))
        psq = ctx.enter_context(tc.tile_pool(name="psq", bufs=2, space="PSUM"))
        pss = ctx.enter_context(tc.tile_pool(name="pss", bufs=2, space="PSUM"))
        psc = ctx.enter_context(tc.tile_pool(name="psc", bufs=2, space="PSUM"))

        wq_sb = wp.tile([128, KT, 128], BF16)
        nc.sync.dma_start(wq_sb[:], wq.ap().rearrange("(kt p) m -> p kt m", p=128))
        wk_sb = wp.tile([128, KT, 128], BF16)
        nc.sync.dma_start(wk_sb[:], wk.ap().rearrange("(kt p) m -> p kt m", p=128))
        wv_sb = wp.tile([128, KT, 128], BF16)
        nc.sync.dma_start(wv_sb[:], wv.ap().rearrange("(kt p) m -> p kt m", p=128))

        # per-batch state, double buffered across batches
        state = {}

        def emit_qk_chunk(b, ch):
            """q+k projections for seq chunk ch (512 wide) of batch b."""
            st = state[b]
            gc = b * CH + ch
            xt_t = xp.tile([128, KT, CW], BF16, tag="xt", name="xt_t")
            nc.sync.dma_start(xt_t[:], xt_v[:, :, gc * CW : (gc + 1) * CW])
            for w_sb, dst in ((wq_sb, st["qT"]), (wk_sb, st["kT"])):
                ps = psq.tile([128, CW], F32, tag="qkv", name="ps_qk")
                for kt in range(KT):
                    nc.tensor.matmul(
                        ps[:],
                        lhsT=w_sb[:, kt],
                        rhs=xt_t[:, kt],
                        start=(kt == 0),
                        stop=(kt == KT - 1),
                    )
                # head A (psum rows 0:64) straight to column block 0
                nc.vector.tensor_copy(
                    out=dst[:, ch * CW : (ch + 1) * CW], in_=ps[0:64, :]
                )
                # head B (rows 64:128) -> staging, then partition-shift DMA
                stg = xp.tile([128, CW], BF16, tag="stg", name="stg")
                nc.vector.tensor_copy(out=stg[64:128, :], in_=ps[64:128, :])
                nc.sync.dma_start(
                    dst[:, S + ch * CW : S + (ch + 1) * CW], stg[64:128, :]
                )

        def emit_v_chunk(b, ch):
            """v projection (natural layout) for chunk ch: 4 seq sub-tiles
            packed into one PSUM bank-chain to amortize entry latency."""
            st = state[b]
            gc = b * CH + ch
            xt_t = xp.tile([128, KT, CW], BF16, tag="xt", name="xt_v_t")
            nc.sync.dma_start(xt_t[:], xt_v[:, :, gc * CW : (gc + 1) * CW])
            ps = psq.tile([128, 512], F32, tag="qkv", name="ps_v")
            for sub in range(4):
                for kt in range(KT):
                    # start clears the whole PSUM bank -> only on the very
                    # first matmul of the bank chain; untouched elements
                    # then overwrite-on-first-write via has_written bits.
                    nc.tensor.matmul(
                        ps[:, sub * 128 : (sub + 1) * 128],
                        lhsT=xt_t[:, kt, sub * 128 : (sub + 1) * 128],
                        rhs=wv_sb[:, kt],
                        start=(sub == 0 and kt == 0),
                        stop=(sub == 3 and kt == KT - 1),
                    )
            nc.vector.tensor_copy(
                out=st["v"][:, ch * 4 : (ch + 1) * 4, :, 0:D],
                in_=ps[:].rearrange("p (s h d) -> p s h d", s=4, h=NH),
            )

        def alloc_batch(b):
            st = {}
            st["qT"] = qkp.tile([64, NH * S], BF16, tag="qT", name="qT")
            st["kT"] = qkp.tile([64, NH * S], BF16, tag="kT", name="kT")
            st["v"] = qkp.tile([128, JT, NH, D + 1], BF16, tag="v", name="v")
            nc.vector.memset(st["v"][:, :, :, D], 1.0)
            state[b] = st

        def emit_scores(p, jt, halves=(0, 1), e=None):
            """Scores + exp for pair p=(b,h), key tile jt. Returns expT."""
            b, h = divmod(p, NH)
            st = state[b]
            if e is None:
                e = ep.tile([128, S], BF16, tag="e", name="e")
            for ihalf in halves:
                ps = pss.tile([128, 1024], F32, tag="s")
                for ic in range(2):
                    i0 = ihalf * 1024 + ic * CW
                    nc.tensor.matmul(
                        ps[:, ic * CW : (ic + 1) * CW],
                        lhsT=st["kT"][:, h * S + jt * 128 : h * S + (jt + 1) * 128],
                        rhs=st["qT"][:, h * S + i0 : h * S + i0 + CW],
                        start=True,
                        stop=True,
                    )
                nc.scalar.activation(
                    e[:, ihalf * 1024 : (ihalf + 1) * 1024],
                    ps[:],
                    mybir.ActivationFunctionType.Exp,
                    scale=0.125,
                )
            return e

        def emit_ctx4(p, it0, exps, o_sb):
            """ctx + Z for four i-tiles it0..it0+3, one PSUM bank chain."""
            b, h = divmod(p, NH)
            st = state[b]
            W = D + 1
            ps = psc.tile([128, 4 * W], F32, tag="c", name="ps_c")
            for jt in range(JT):
                for q in range(4):
                    # bank-wide clear on start -> first matmul only
                    nc.tensor.matmul(
                        ps[:, q * W : (q + 1) * W],
                        lhsT=exps[jt][:, (it0 + q) * 128 : (it0 + q + 1) * 128],
                        rhs=st["v"][:, jt, h],
                        start=(jt == 0 and q == 0),
                        stop=(jt == JT - 1 and q == 3),
                    )
            rz = zp.tile([128, 4], F32, tag="rz", name="rz")
            z_view = ps[:].rearrange("p (q w) -> p q w", w=W)[:, :, D]
            nc.vector.reciprocal(rz[:], z_view)
            for q in range(4):
                nc.vector.tensor_scalar_mul(
                    o_sb[:, it0 + q], ps[:, q * W : q * W + D], rz[:, q : q + 1]
                )

        def emit_out_dma(p, o_sb):
            b, h = divmod(p, NH)
            nc.sync.dma_start(out_v[:, b, :, h * D : (h + 1) * D], o_sb[:])

        # ---- prologue: ACT table warmup + batch-0 projections with early
        # partial scores (i-half 0 of jt 0-3 only needs q chunks 0-1 + k ch0)
        warm_src = wp.tile([128, 1], F32, name="warm_src")
        warm_dst = wp.tile([128, 1], F32, name="warm_dst")
        nc.vector.memset(warm_src[:], 0.0)
        nc.scalar.activation(
            warm_dst[:], warm_src[:], mybir.ActivationFunctionType.Exp
        )
        alloc_batch(0)
        emit_qk_chunk(0, 0)
        emit_qk_chunk(0, 1)
        exps0 = [emit_scores(0, jt, halves=(0,)) for jt in range(4)]
        emit_qk_chunk(0, 2)
        emit_qk_chunk(0, 3)
        for jt in range(4):
            emit_scores(0, jt, halves=(1,), e=exps0[jt])
        prev = (0, exps0)

        # ---- pipelined pairs ----
        # per pair: 16 scores slots; ctx of the previous pair as four
        # 4-i-tile chains at jt 0,2,4,6 (expT released by jt 6); projection
        # work for upcoming batches spread over jt 8,10,12,14.
        NP = B * NH
        for p in range(NP):
            b, h = divmod(p, NH)
            if p == 0:
                exps = exps0
                jts = range(4, JT)
                o_prev = None
            else:
                exps = []
                jts = range(JT)
                o_prev = op.tile([128, IT, D], F32, tag="o", name="o_sb")
            for jt in jts:
                exps.append(emit_scores(p, jt))
                if p > 0:
                    if jt in (0, 2, 4, 6):
                        emit_ctx4(prev[0], 2 * jt, prev[1], o_prev)
                    elif jt == 7:
                        emit_out_dma(prev[0], o_prev)
                if jt in (8, 10, 12, 14):
                    if p == 0 or (h == 0 and b >= 1):
                        emit_v_chunk(b, (jt - 8) // 2)  # current batch's v
                    elif h == 1 and b + 1 < B:
                        if jt == 8:
                            alloc_batch(b + 1)
                        emit_qk_chunk(b + 1, (jt - 8) // 2)
            prev = (p, exps)

        # ---- epilogue: ctx for the last pair ----
        o_last = op.tile([128, IT, D], F32, tag="o", name="o_last")
        for it0 in range(0, IT, 4):
            emit_ctx4(prev[0], it0, prev[1], o_last)
        emit_out_dma(prev[0], o_last)

    nc.compile()
    return nc


def _get_nc():
    if "nc" not in _CACHE:
        _CACHE["nc"] = _build()
    return _CACHE["nc"]


def kernel(
    query,
    key=None,
    attention_mask=None,
    Wq=None,
    bq=None,
    Wk=None,
    bk=None,
    Wv=None,
    bv=None,
    seq_length=2048,
    **_unused,
):
    query = np.asarray(query)
    Wq = np.asarray(Wq)
    Wk = np.asarray(Wk)
    Wv = np.asarray(Wv)
    if attention_mask is not None and not np.all(np.asarray(attention_mask) == 1):
        raise NotImplementedError("kernel assumes an all-ones attention mask")
    for bias in (bq, bk, bv):
        if bias is not None and np.any(np.asarray(bias)):
            raise NotImplementedError("kernel assumes zero biases")

    x = query.reshape(-1, HID)  # [8192, 1024]
    xt = np.ascontiguousarray(x.T).astype(ml_dtypes.bfloat16)  # [1024, 8192]

    in_maps = []
    for c in range(NCORES):
        cols = slice(c * 128, (c + 1) * 128)
        in_maps.append(
            {
                "xt": xt,
                "wq": np.ascontiguousarray(Wq[:, cols]).astype(ml_dtypes.bfloat16),
                "wk": np.ascontiguousarray(Wk[:, cols]).astype(ml_dtypes.bfloat16),
                "wv": np.ascontiguousarray(Wv[:, cols]).astype(ml_dtypes.bfloat16),
            }
        )

    nc = _get_nc()
    res = run_bass_kernel_spmd(
        nc,
        in_maps,
        core_ids=list(range(NCORES)),
        trace=bool(_CACHE.get("trace", False)),
    )
    _CACHE["last_result"] = res
    out = np.concatenate(
        [res.results[c]["out"] for c in range(NCORES)], axis=1
    ).astype(np.float32)
    return out
